# revision 56
# baseline (speedup 1.0000x reference)
"""Trainium2 Bass kernel: causal attention (QKV projection + causal softmax + AV).

Problem: x[4, 4096, 768] fp32, per-head projections to d=64, full causal
attention per batch, output [4, 4096, 64] fp32.

Sharding: 8 cores = 4 batches x 2 parity groups. Core (b, j) computes the
output rows of batch b whose 128-row block index i satisfies i % 2 == j.
One uniform SPMD program: for j=0 cores the host shifts x down by one
128-row block (prepending zeros), which makes the causal structure of both
parities identical in device coordinates (device q-blocks are always the odd
blocks 1,3,...,31; k-slot g holds true block g-1 for j=0 and g for j=1; the
dead slot 0 of j=0 is zeroed through the V' ones-column, sent per-core).

Math shortcuts: bk is dropped (adds a per-row constant to scores ->
softmax-invariant); bv is added on the host after normalization (softmax
weights sum to 1). Only bq is applied on device.

I/O: every DMA issue is chained by the tile framework to the previous
DMA's completion EXCEPT consecutive same-kind same-queue DMAs, which
pipeline back-to-back on the (serial) DMA-engine device of the cost
model. So the host pre-transposes x and all loads are plain strided
DMACopies on the single SP queue, ordered by first use (weights, x chunk
0 in halves interleaved with the weight halves, wq, consts, x chunks
1..7); piece sizes stay >= the 650ns issue cadence.

Device pipeline per core (all matmuls bf16, fp32 PSUM accumulation):
  proj passB: [wv|wk] stationary -> PSUM (V rows 0:64 / K rows 64:128),
     one DVE copy drops both into the kvt tile where each is used; V' is
     PE-transposed from kvt into vs = [V | 1] per k-slot.
  proj passA: wq stationary -> Q^T in PSUM partitions 64:128, + bq (DVE).
  attn (per 512-col q chunk c, k-slot pairs): two matmuls K^T_g.T @ Q^T
     into a [128, 1024] PSUM tile (column-trimmed to the causal extent);
     exp on ACT (scale 1/8, bf16 out, one instruction per pair - a strided
     AP covers both live regions of diagonal pairs); causal-diagonal mask
     mul on DVE; AV accumulates V'.T @ P^T into a [65, 512] PSUM tile
     whose row 64 is the softmax denominator (ones column of V').
  schedule: scores run one pair ahead of AV; projection matmuls are
     interleaved as fillers (deadline-enforced); seg0's first two pairs
     are scored in q-column halves so chunk-0 work starts before chunk 1
     lands; segs 2 and 3 are emitted as two alternating streams, which
     doubles the score-buffer rotation interval and hides the exp->scores
     semaphore latency; seg3's final AV closes in column halves so most
     of the last output stores before the final pairs finish.
The unnormalized [65, 512] bf16 tiles go to DRAM; the host divides by the
denominator row, adds bv, and transposes.
"""

import numpy as np
import ml_dtypes
from contextlib import ExitStack

import concourse.bass as bass
import concourse.mybir as mybir
import concourse.tile as tile
from concourse import bacc
from concourse.bass_utils import run_bass_kernel_spmd

F32 = mybir.dt.float32
BF16 = mybir.dt.bfloat16

SEQ = 4096
DIN = 768
DOUT = 64
NCC = DIN // 128          # 6 contraction chunks
NSC = SEQ // 512          # 8 seq chunks (projection granularity)
NBLK = SEQ // 128         # 32 k-slots
NQC = 4                   # q chunks of 512 local columns (2048 own q rows)
SCALE = 1.0 / 8.0
EXPF = mybir.ActivationFunctionType.Exp

_CACHED_NC = None


def build_nc(repeats=1, pp=(8.0, 4.0, 1.5, 0.8), SOLO2=3):
    nc = bacc.Bacc("TRN2", target_bir_lowering=False, debug=False)

    # x pre-transposed on host: [768, 4096] so every load is a plain
    # strided DMA (cheaper than XBAR transposes on the serial DMA device)
    xT = nc.dram_tensor("xT", [DIN, SEQ], BF16, kind="ExternalInput")
    wA = nc.dram_tensor("wA", [DIN, 192], BF16, kind="ExternalInput")  # [wv|wk|wq]
    mA = nc.dram_tensor("mA", [128, 592], BF16, kind="ExternalInput")
    o = nc.dram_tensor("o", [NQC, 65, 512], BF16, kind="ExternalOutput")

    with tile.TileContext(nc) as tc, ExitStack() as ctx:
        cpool = ctx.enter_context(tc.tile_pool(name="const", bufs=1))
        ptp = ctx.enter_context(tc.tile_pool(name="pt", bufs=4))
        ocp = ctx.enter_context(tc.tile_pool(name="oc", bufs=2))
        psproj = ctx.enter_context(tc.tile_pool(name="psproj", bufs=2, space="PSUM"))
        psst = ctx.enter_context(tc.tile_pool(name="psst", bufs=2, space="PSUM"))
        psav = ctx.enter_context(tc.tile_pool(name="psav", bufs=2, space="PSUM"))

        wsb = cpool.tile([128, NCC * 192], BF16)    # [wv|wk|wq] per cc chunk
        mc = cpool.tile([128, 592], BF16)           # mask | idn | bq | dead
        kvt = cpool.tile([128, NSC * 512], BF16)    # K^T rows 64:128, V^T rows 0:64
        xtf = cpool.tile([128, NSC * NCC * 512], BF16)  # x^T, whole sequence
        qt = cpool.tile([128, 16 * 128], BF16)      # Q^T own blocks, rows 64:128
        vs = cpool.tile([128, NBLK * 65], BF16)     # V' = [V | 1] per k-slot

        # ---- DMA issue plan: spread across 4 DGE queues so transfers
        # overlap and the first x^T chunk lands ~2us in.
        xT3 = xT.rearrange("(cc p) s -> p cc s", p=128)

        def trans(sc, eng, half=None, nsplit=2):
            """Plain strided load of x^T chunk sc (or a 1/nsplit piece)."""
            if half is None:
                c0, c1 = 0, 512
            else:
                w = 512 // nsplit
                c0, c1 = half * w, half * w + w
            s0 = sc * 512
            eng.dma_start(
                xtf[:, sc * NCC * 512:(sc + 1) * NCC * 512]
                .rearrange("p (cc s) -> p cc s", cc=NCC)[:, :, c0:c1],
                xT3[:, :, s0 + c0:s0 + c1],
            )

        # All loads are DmaTransposeAnt on the single sync (SP) queue: the
        # tile framework chains every DMA issue to the previous DMA's
        # completion EXCEPT consecutive same-kind same-queue DMAs, which
        # pipeline back-to-back on the (serial) DMA engine device.
        wv3 = wsb[:].rearrange("p (cc m) -> p cc m", cc=NCC)
        wA3 = wA.rearrange("(cc p) m -> p cc m", p=128)
        nc.sync.dma_start(wv3[:, 0:3, 0:128], wA3[:, 0:3, 0:128])
        trans(0, nc.sync, half=0)
        nc.sync.dma_start(wv3[:, 3:NCC, 0:128], wA3[:, 3:NCC, 0:128])
        trans(0, nc.sync, half=1)
        nc.sync.dma_start(wv3[:, :, 128:192], wA3[:, :, 128:192])
        nc.sync.dma_start(mc[:], mA[:, :])
        trans(1, nc.sync, half=0)
        trans(1, nc.sync, half=1)
        for _sc in range(2, NSC):
            trans(_sc, nc.sync)
        # f32 copy of bq (tensor_scalar needs f32 scalars)
        bqf = cpool.tile([128, 2], F32)
        nc.gpsimd.tensor_copy(bqf[:], mc[:, 576:578])
        # ones column of V'; slot 0 gets a per-core 0/1 (mc col 578) which
        # zeroes the j=0 dead slot's numerator AND denominator contribution
        nc.vector.memset(
            vs[:].rearrange("p (g e) -> p g e", g=NBLK)[:, :, 64:65], 1.0
        )
        nc.vector.tensor_copy(
            vs[:].rearrange("p (g e) -> p g e", g=NBLK)[:, 0:1, 64:65],
            mc[:, 578:579],
        )

        def xts(sc, cc):
            base = sc * NCC * 512 + cc * 512
            return xtf[:, base:base + 512]

        # ---- projection emitters; each returns a list of closures, one
        # per PE instruction (posts ride on the closure that needs them).
        proj_state = {}

        def passB_units(sc, split=False):
            """K^T rows 64:128 and V^T rows 0:64, [wv|wk] stationary."""
            units = []

            def mk(cc, c0, cols, start, stop):
                def f():
                    if cc == 0 and c0 == 0:
                        proj_state[("kp", sc)] = psproj.tile(
                            [128, 512], F32, tag="proj", name="kp")
                    kp = proj_state[("kp", sc)]
                    nc.tensor.matmul(
                        kp[:, c0:c0 + cols],
                        wsb[:, cc * 192:cc * 192 + 128],
                        xts(sc, cc)[:, c0:c0 + cols],
                        start=start, stop=stop,
                    )
                    if stop and c0 + cols == 512:
                        kp = proj_state.pop(("kp", sc))
                        # one copy: K rows 64:128 (no bias; bk is softmax-
                        # invariant) and V rows 0:64 land where each is used
                        nc.vector.tensor_copy(
                            kvt[:, sc * 512:(sc + 1) * 512], kp[:]
                        )
                return f

            if split:
                w = 512 // split
                for c0 in range(0, 512, w):
                    for cc in range(NCC):
                        units.append((("B", sc), mk(cc, c0, w, cc == 0, cc == NCC - 1)))
            else:
                for cc in range(NCC):
                    units.append((("B", sc), mk(cc, 0, 512, cc == 0, cc == NCC - 1)))
            return units

        def vtrans_units(sc):
            """V' blocks via PE transpose of vt, then one DVE copy to vs."""
            units = []

            def mk(t):
                def f():
                    if t == 0:
                        proj_state[("vp", sc)] = psproj.tile(
                            [128, 256], BF16, tag="proj", name="vp")
                    vp = proj_state[("vp", sc)]
                    nc.tensor.transpose(
                        vp[:, t * 64:(t + 1) * 64],
                        kvt[0:64, sc * 512 + t * 128: sc * 512 + (t + 1) * 128],
                        mc[0:64, 512:576],
                    )
                    if t == 3:
                        vp = proj_state.pop(("vp", sc))
                        nc.vector.tensor_copy(
                            vs[:].rearrange("p (g e) -> p g e", g=NBLK)[
                                :, sc * 4:(sc + 1) * 4, 0:64
                            ],
                            vp[:].rearrange("p (g e) -> p g e", g=4),
                        )
                return f

            for t in range(4):
                units.append((("Vt", sc), mk(t)))
            return units

        def passA_units(sc):
            """Q^T for own (odd) q-blocks of this chunk, into rows 64:128."""
            units = []

            def mk(cc):
                def f():
                    if cc == 0:
                        proj_state[("qp", sc)] = psproj.tile(
                            [128, 256], F32, tag="proj", name="qp")
                    qp = proj_state[("qp", sc)]
                    rhs = (
                        xts(sc, cc)
                        .rearrange("p (a b s) -> p a b s", a=2, b=2)[:, :, 1, :]
                    )
                    nc.tensor.matmul(
                        qp[64:128, :], wsb[:, cc * 192 + 128:cc * 192 + 192], rhs,
                        start=(cc == 0), stop=(cc == NCC - 1),
                    )
                    if cc == NCC - 1:
                        qp = proj_state.pop(("qp", sc))
                        nc.vector.tensor_scalar_add(
                            qt[64:128, sc * 256:(sc + 1) * 256],
                            qp[64:128, :], bqf[64:128, 0:1],
                        )
                return f

            for cc in range(NCC):
                units.append((("A", sc), mk(cc)))
            return units

        # ---- attention ----
        def slot_geom(c, g):
            s = g - (8 * c + 1)
            if s < 1:
                return 0, 512
            off = 128 * ((s + 1) // 2)
            return off, 512 - off

        parts = {}

        def attn_seg(c, p_lo, p_hi, final, fillers, per_pair):
            for _ in attn_seg_gen(c, p_lo, p_hi, final, fillers, per_pair):
                pass

        def attn_seg_gen(c, p_lo, p_hi, final, fillers, per_pair):
            """Seg c pairs [p_lo, p_hi): scores one pair ahead of AV.

            fillers: ordered [(key, fn)] of projection units interleaved for
            PE occupancy; units a pair depends on are force-drained first.
            Non-final ranges park their partial AV in SBUF (parts).
            """
            npairs = 4 * c + 4
            av = psav.tile([65, 512], F32, tag="av", name="av")
            pending = []  # [(p, pt, geom)] awaiting AV

            def ensure(keys):
                while any(k in keys for k, _ in fillers):
                    _, fn = fillers.pop(0)
                    fn()

            def emit_scores(p):
                g0, g1 = 2 * p, 2 * p + 1
                off0, w0 = slot_geom(c, g0)
                off1, w1 = slot_geom(c, g1)
                st = psst.tile([128, 1024], F32, tag="st", name="st")
                if c == 0 and p < 2:
                    # the seg's second Q chunk lands late: columns [0:256]
                    # (first chunk's Q) can score and exp before the second
                    # passA completes; respects the compacted score layout
                    pt = ptp.tile([128, 1024], BF16, name="pt")
                    for cl, ch in ((0, 256), (256, 512)):
                        for g, base, off in ((g0, 0, off0), (g1, 512, off1)):
                            a = max(cl, off)
                            if a >= ch:
                                continue
                            nc.tensor.matmul(
                                st[:, base + a - off:base + ch - off],
                                kvt[64:128, g * 128:(g + 1) * 128],
                                qt[64:128, c * 512 + a:c * 512 + ch],
                                start=True, stop=True,
                            )
                            nc.scalar.activation(
                                pt[:, base + a - off:base + ch - off],
                                st[:, base + a - off:base + ch - off],
                                EXPF, bias=0.0, scale=SCALE)
                    if p >= npairs - 4:
                        nc.vector.tensor_mul(
                            pt[:, 512:512 + w1], pt[:, 512:512 + w1],
                            mc[:, 0:w1]
                        )
                    pending.append((p, pt, (off0, w0, off1, w1)))
                    return
                nc.tensor.matmul(
                    st[:, 0:w0], kvt[64:128, g0 * 128:(g0 + 1) * 128],
                    qt[64:128, c * 512 + off0: c * 512 + off0 + w0],
                    start=True, stop=True,
                )
                nc.tensor.matmul(
                    st[:, 512:512 + w1], kvt[64:128, g1 * 128:(g1 + 1) * 128],
                    qt[64:128, c * 512 + off1: c * 512 + off1 + w1],
                    start=True, stop=True,
                )
                pt = ptp.tile([128, 1024], BF16, name="pt")
                if w0 == 512:
                    nc.scalar.activation(pt[:, 0:512 + w1], st[:, 0:512 + w1],
                                         EXPF, bias=0.0, scale=SCALE)
                else:
                    # diagonal pairs have w0 == w1: one strided-AP exp covers
                    # both live regions [0:w0] and [512:512+w1]
                    sv = st[:].rearrange("p (j w) -> p j w", j=2)[:, :, 0:w0]
                    pv = pt[:].rearrange("p (j w) -> p j w", j=2)[:, :, 0:w0]
                    nc.scalar.activation(pv, sv, EXPF, bias=0.0, scale=SCALE)
                if p >= npairs - 4:
                    # odd member of the last four pairs is causal-diagonal
                    nc.vector.tensor_mul(
                        pt[:, 512:512 + w1], pt[:, 512:512 + w1], mc[:, 0:w1]
                    )
                pending.append((p, pt, (off0, w0, off1, w1)))

            splitL = final and c == 3 and p_hi == npairs
            started = set()

            def emit_av(first):
                p, pt, (off0, w0, off1, w1) = pending.pop(0)
                for g, off, base in ((2 * p, off0, 0),
                                     (2 * p + 1, off1, 512)):
                    vsl = vs[:, g * 65:(g + 1) * 65]
                    if not splitL:
                        nc.tensor.matmul(
                            av[:, off:512], vsl, pt[:, base:base + 512 - off],
                            start=first and base == 0,
                            stop=(p == p_hi - 1 and base == 512),
                        )
                        continue
                    # L=[0:256] closes at pair 13 (slot 27); R at pair 15
                    pieces = []
                    if off < 256:
                        pieces.append(("L", off, 256, g == 27))
                    pieces.append(("R", max(off, 256), 512, g == 31))
                    for grp, o0, o1, stop in pieces:
                        nc.tensor.matmul(
                            av[:, o0:o1], vsl,
                            pt[:, base + o0 - off: base + o1 - off],
                            start=grp not in started, stop=stop,
                        )
                        started.add(grp)
                        if stop and grp == "L":
                            ocl = ocp.tile([65, 256], BF16, name="ocl")
                            nc.vector.tensor_copy(ocl[:], av[:, 0:256])
                            nc.sync.dma_start(o[c, :, 0:256], ocl[:])

            budget = 0.0
            for p in range(p_lo, p_hi):
                need = {("B", (2 * p + 1) // 4), ("Vt", (2 * p + 1) // 4)}
                if p == p_lo:
                    need |= {("A", 2 * c), ("A", 2 * c + 1)}
                ensure(need)
                emit_scores(p)
                budget += per_pair
                while fillers and budget >= 1.0:
                    _, fn = fillers.pop(0)
                    fn()
                    budget -= 1.0
                if p >= p_lo + 1:
                    emit_av(p == p_lo + 1)
                yield
            emit_av(p_hi == p_lo + 1)
            if splitL:
                oc = ocp.tile([65, 256], BF16, name="oc")
                nc.vector.tensor_copy(oc[:], av[:, 256:512])
                nc.sync.dma_start(o[c, :, 256:512], oc[:])
            elif final:
                oc = ocp.tile([65, 512], BF16, name="oc")
                if c in parts:
                    nc.vector.tensor_add(oc[:], av[:], parts.pop(c)[:])
                else:
                    nc.vector.tensor_copy(oc[:], av[:])
                nc.sync.dma_start(o[c, :, :], oc[:])
            else:
                part = ocp.tile([65, 512], F32, tag="part", name="part")
                nc.vector.tensor_copy(part[:], av[:])
                parts[c] = part

        for _rep in range(repeats):
            # prologue: chunks 0,1 projected up front (B0 split for latency)
            for _, fn in (passB_units(0, split=2) + passA_units(0)
                          + vtrans_units(0) + passB_units(1, split=2)
                          + passA_units(1) + vtrans_units(1)):
                fn()

            f0 = (passB_units(2) + vtrans_units(2) + passA_units(2)
                  + passB_units(3) + vtrans_units(3) + passA_units(3))
            attn_seg(0, 0, 4, True, f0, per_pair=pp[0])
            f1 = f0
            f1x = (f1 + passB_units(4) + vtrans_units(4) + passA_units(4)
                   + passB_units(5) + vtrans_units(5) + passA_units(5))
            attn_seg(1, 0, 8, True, f1x, pp[1])
            f23 = (f1x + passA_units(6) + passA_units(7)
                   + passB_units(6) + vtrans_units(6))
            b7units = passB_units(7) + vtrans_units(7)
            g2 = attn_seg_gen(2, 0, 12, True, f23, pp[2])
            g3 = attn_seg_gen(3, 0, 16, True, f23, pp[3])

            def step(g):
                try:
                    next(g)
                    return True
                except StopIteration:
                    return False

            for _ in range(SOLO2):
                step(g2)
            # alternate the two segs: doubles the effective score-buffer
            # rotation depth, hiding the exp->scores semaphore latency
            a2 = a3 = True
            while a2:
                if a3:
                    a3 = step(g3)
                a2 = step(g2)
            # B7/Vt7 reserved as fillers for seg3's solo tail
            f23.extend(b7units)
            while a3:
                a3 = step(g3)
            for _, fn in f23:
                fn()

    nc.compile()
    return nc


def _get_nc():
    global _CACHED_NC
    if _CACHED_NC is None:
        _CACHED_NC = build_nc()
    return _CACHED_NC


def _host_inputs(x, wq, bq, wk, bk, wv, bv):
    bf = ml_dtypes.bfloat16
    wA = np.ascontiguousarray(
        np.concatenate([wv, wk, wq], axis=1)
    ).astype(bf)  # [768, 192]
    xbf = np.ascontiguousarray(x).astype(bf)

    def mconst(live0):
        m = np.zeros((128, 592), np.float32)
        m[:, 0:128] = np.triu(np.ones((128, 128), np.float32))
        m[:, 128:512] = 1.0
        m[0:64, 512:576] = np.eye(64, dtype=np.float32)
        m[64:128, 576] = bq
        m[:, 578] = live0  # V' slot-0 ones column: 0 kills j=0's dead slot
        return np.ascontiguousarray(m).astype(bf)  # [128, 592]

    mT0, mT1 = mconst(0.0), mconst(1.0)
    in_maps = []
    for core in range(8):
        b, j = core // 2, core % 2
        if j == 0:
            xdev = np.concatenate(
                [np.zeros((128, DIN), bf), xbf[b][: SEQ - 128]], axis=0
            )
        else:
            xdev = xbf[b]
        in_maps.append({
            "xT": np.ascontiguousarray(xdev.T),
            "wA": wA, "mA": mT0 if j == 0 else mT1,
        })
    return in_maps


def _assemble(results, bv):
    out = np.empty((4, SEQ, DOUT), np.float32)
    for core in range(8):
        b, j = core // 2, core % 2
        od = results[core]["o"]  # [NQC, 65, 512]
        for c in range(NQC):
            num = od[c, 0:64, :].astype(np.float64)
            den = od[c, 64, :].astype(np.float64)
            oc = (num / den).T.astype(np.float32) + bv  # [512, 64]
            for t in range(4):
                r0 = (8 * c + 2 * t + j) * 128
                out[b, r0:r0 + 128] = oc[t * 128:(t + 1) * 128]
    return out


def kernel(x, wq, bq, wk, bk, wv, bv):
    x = np.asarray(x, dtype=np.float32)
    args = [np.asarray(a, dtype=np.float32) for a in (wq, bq, wk, bk, wv, bv)]
    nc = _get_nc()
    in_maps = _host_inputs(x, *args)
    br = run_bass_kernel_spmd(nc, in_maps, core_ids=list(range(8)))
    return _assemble(br.results, args[5])
